# revision 8
# baseline (speedup 1.0000x reference)
"""Sparse multi-head attention (nn_MultiHeadAttention_44332652429419) on 8 trn2 cores.

Strategy (tensor-parallel over H=16 heads, 2 heads per core):
  Host: compose the two stacked linear layers (q/k/v_proj followed by
        MultiheadAttention in_proj) into one weight per tensor; build the
        dense multiplicative mask exp(additive_mask) transposed; transpose x.
  Device (per core, SPMD with per-core weight slices):
    q2T/k2T/v2T = W_c @ x.T + b_c           [128, 3072] (2 heads x 64 dims)
    scoresT[mk,nq] = k2T_h.T-slice @ q2T_h  (K=64, two heads row-packed in PE)
    P = exp(scoresT * 1/8) * maskT          (ACT exp from PSUM, DVE multiply)
    outT_aug = [v_h | 1].T @ P              (rowsum via ones-augmented V)
    attnT = outT[:64] / outT[64]            (DVE recip + partition broadcast)
    ypart = attnT.T-slices @ woT_c          (partial out_proj, K=128)
  Host: y = sum_c ypart_c + bo
"""
import os
import sys

sys.path.insert(0, "/opt/trn_rl_repo")

import numpy as np
from contextlib import ExitStack

import concourse.bass as bass
import concourse.bacc as bacc
import concourse.mybir as mybir
import concourse.tile as tile
from concourse.bass_utils import run_bass_kernel_spmd
from concourse.masks import make_identity

F32 = mybir.dt.float32
F32R = mybir.dt.float32r
BF16 = mybir.dt.bfloat16
AF = mybir.ActivationFunctionType
ALU = mybir.AluOpType

N = 3072
IN_F = 1024
OUT_F = 1024
H = 16
D = 64
NCORES = 8
HPC = H // NCORES            # heads per core = 2
CW = HPC * D                 # per-core width = 128
P = 128
NT = N // P                  # 24 node tiles
KT = IN_F // P               # 8 contraction tiles
SQ = 1024                    # query strip width (phase B)
NSQ = N // SQ                # 3 strips
SP = 512                     # proj strip width (phase A)
NSP = N // SP                # 6 strips
SCALE = 1.0 / 8.0            # 1/sqrt(D)

MASK_DT = F32                # mask dtype on device


def build_program():
    nc = bacc.Bacc()
    xT = nc.declare_dram_parameter("xT", [IN_F, N], F32R, isOutput=False)
    maskT = nc.declare_dram_parameter("maskT", [N, N], MASK_DT, isOutput=False)
    wqT = nc.declare_dram_parameter("wqT", [IN_F, CW], F32R, isOutput=False)
    wkT = nc.declare_dram_parameter("wkT", [IN_F, CW], F32R, isOutput=False)
    wvT = nc.declare_dram_parameter("wvT", [IN_F, CW], F32R, isOutput=False)
    bq = nc.declare_dram_parameter("bq", [CW], F32, isOutput=False)
    bk = nc.declare_dram_parameter("bk", [CW], F32, isOutput=False)
    bv = nc.declare_dram_parameter("bv", [CW], F32, isOutput=False)
    woT = nc.declare_dram_parameter("woT", [CW, OUT_F], F32R, isOutput=False)
    ypart = nc.declare_dram_parameter("ypart", [N, OUT_F], F32, isOutput=True)

    with tile.TileContext(nc) as tc, ExitStack() as ctx:
        cst = ctx.enter_context(tc.tile_pool(name="cst", bufs=1))

        ident = cst.tile([P, P], F32)
        make_identity(nc, ident)

        # persistent SBUF tensors
        q2T = cst.tile([P, N], F32R)
        k2T = cst.tile([P, N], F32R)
        attnT = cst.tile([P, N], F32R)
        vaug = [cst.tile([P, NT, D + 1], F32R, tag=f"vaug{h}", name=f"vaug{h}")
                for h in range(HPC)]
        ones_col = cst.tile([P, 1], F32)
        nc.vector.memset(ones_col[:], 1.0)
        for h in range(HPC):
            nc.vector.tensor_copy(vaug[h][:, :, D:D + 1],
                                  ones_col[:, 0:1, None].to_broadcast([P, NT, 1]))

        # weights
        wq_sb = cst.tile([P, KT, CW], F32R)
        nc.sync.dma_start(wq_sb[:], wqT.rearrange("(k p) m -> p k m", p=P))
        wk_sb = cst.tile([P, KT, CW], F32R)
        nc.sync.dma_start(wk_sb[:], wkT.rearrange("(k p) m -> p k m", p=P))
        wv_sb = cst.tile([P, KT, CW], F32R)
        nc.sync.dma_start(wv_sb[:], wvT.rearrange("(k p) m -> p k m", p=P))
        wo_sb = cst.tile([P, OUT_F], F32R)
        nc.sync.dma_start(wo_sb[:], woT[:])
        bq_sb = cst.tile([P, 1], F32)
        nc.sync.dma_start(bq_sb[:], bq[:, None])
        bk_sb = cst.tile([P, 1], F32)
        nc.sync.dma_start(bk_sb[:], bk[:, None])
        bv_sb = cst.tile([P, 1], F32)
        nc.sync.dma_start(bv_sb[:], bv[:, None])

        # ---- Phase A: projections ----
        with tc.tile_pool(name="pa_sb", bufs=2) as pa_sb, \
             tc.tile_pool(name="pa_ps", bufs=2, space="PSUM") as pa_ps:
            for s in range(NSP):
                sl = slice(s * SP, (s + 1) * SP)
                xs = pa_sb.tile([P, KT, SP], F32R, tag="xs")
                nc.sync.dma_start(
                    xs[:], xT.rearrange("(k p) n -> p k n", p=P)[:, :, sl])
                for w_sb, b_sb, dest in ((wq_sb, bq_sb, q2T), (wk_sb, bk_sb, k2T)):
                    ps = pa_ps.tile([P, SP], F32, tag="ps_proj")
                    for k in range(KT):
                        nc.tensor.matmul(ps[:], w_sb[:, k, :], xs[:, k, :],
                                         start=(k == 0), stop=(k == KT - 1))
                    nc.scalar.activation(dest[:, sl], ps[:], AF.Identity,
                                         bias=b_sb[:, 0:1])
                # v: project then transpose into vaug
                ps = pa_ps.tile([P, SP], F32, tag="ps_proj")
                for k in range(KT):
                    nc.tensor.matmul(ps[:], wv_sb[:, k, :], xs[:, k, :],
                                     start=(k == 0), stop=(k == KT - 1))
                v2Ts = pa_sb.tile([P, SP], F32, tag="v2Ts")
                nc.scalar.activation(v2Ts[:], ps[:], AF.Identity, bias=bv_sb[:, 0:1])
                for b in range(SP // P):
                    t = s * (SP // P) + b
                    ps_t = pa_ps.tile([P, P], F32, tag="ps_t")
                    nc.tensor.transpose(ps_t[:], v2Ts[:, b * P:(b + 1) * P], ident[:])
                    for h in range(HPC):
                        nc.any.tensor_copy(vaug[h][:, t, 0:D],
                                           ps_t[:, h * D:(h + 1) * D])

        # ---- Phase B: attention ----
        with tc.tile_pool(name="pb_sb", bufs=3) as pb_sb, \
             tc.tile_pool(name="pb_ps", bufs=2, space="PSUM") as pb_ps, \
             tc.tile_pool(name="pb_pso", bufs=1, space="PSUM") as pb_pso:
            for sq in range(NSQ):
                qsl = slice(sq * SQ, (sq + 1) * SQ)
                ps_o = [pb_pso.tile([D + 1, SQ], F32, tag=f"ps_o{h}", name=f"ps_o{h}")
                        for h in range(HPC)]
                for mk in range(NT):
                    mt = pb_sb.tile([P, SQ], MASK_DT, tag="mt")
                    nc.sync.dma_start(mt[:], maskT[mk * P:(mk + 1) * P, qsl])
                    for h in range(HPC):
                        hsl = slice(h * D, (h + 1) * D)
                        ps_s = pb_ps.tile([P, SQ], F32, tag="ps_s")
                        for half in range(SQ // 512):
                            nc.tensor.matmul(
                                ps_s[:, half * 512:(half + 1) * 512],
                                k2T[hsl, mk * P:(mk + 1) * P],
                                q2T[hsl, sq * SQ + half * 512:
                                    sq * SQ + (half + 1) * 512],
                                start=True, stop=True,
                                tile_position=(h * D, 0),
                            )
                        p = pb_sb.tile([P, SQ], F32, tag="p")
                        nc.scalar.activation(p[:], ps_s[:], AF.Exp, scale=SCALE)
                        pm = pb_sb.tile([P, SQ], F32R, tag="pm")
                        nc.vector.tensor_tensor(pm[:], p[:], mt[:], ALU.mult)
                        for half in range(SQ // 512):
                            nc.tensor.matmul(
                                ps_o[h][:, half * 512:(half + 1) * 512],
                                vaug[h][:, mk, :],
                                pm[:, half * 512:(half + 1) * 512],
                                start=(mk == 0), stop=(mk == NT - 1),
                            )
                for h in range(HPC):
                    recip = pb_sb.tile([1, SQ], F32, tag="recip")
                    nc.vector.reciprocal(recip[:], ps_o[h][D:D + 1, :])
                    bc = pb_sb.tile([D, SQ], F32, tag="bc")
                    nc.gpsimd.partition_broadcast(bc[:], recip[:])
                    nc.vector.tensor_tensor(attnT[h * D:(h + 1) * D, qsl],
                                            ps_o[h][0:D, :], bc[:], ALU.mult)

        # ---- Phase C: partial out_proj ----
        with tc.tile_pool(name="pc_sb", bufs=3) as pc_sb, \
             tc.tile_pool(name="pc_ps", bufs=3, space="PSUM") as pc_ps:
            for t in range(NT):
                for f in range(OUT_F // 512):
                    ps_y = pc_ps.tile([P, 512], F32, tag="ps_y")
                    nc.tensor.matmul(ps_y[:], attnT[:, t * P:(t + 1) * P],
                                     wo_sb[:, f * 512:(f + 1) * 512],
                                     start=True, stop=True)
                    ys = pc_sb.tile([P, 512], F32, tag="ys")
                    nc.any.tensor_copy(ys[:], ps_y[:])
                    nc.sync.dma_start(
                        ypart[t * P:(t + 1) * P, f * 512:(f + 1) * 512], ys[:])

    nc.compile()
    return nc


_PROGRAM = None
LAST_RESULTS = None


def _get_program():
    global _PROGRAM
    if _PROGRAM is None:
        _PROGRAM = build_program()
    return _PROGRAM


def _softplus(x):
    x = np.asarray(x, np.float32)
    return np.logaddexp(0.0, x).astype(np.float32)


def host_prep(inputs):
    x = np.asarray(inputs["x"], np.float32)
    edge_index = np.asarray(inputs["edge_index"])
    edge_type = np.asarray(inputs["edge_type"])
    etw = np.asarray(inputs["edge_type_weights"], np.float32)

    def f32(k):
        return np.asarray(inputs[k], np.float32)

    # compose the two linear layers: q2 = x @ (wiq@wq).T + (wiq@bq + biq)
    WQ = f32("wiq") @ f32("wq")
    bQ = f32("wiq") @ f32("bq") + f32("biq")
    WK = f32("wik") @ f32("wk")
    bK = f32("wik") @ f32("bk") + f32("bik")
    WV = f32("wiv") @ f32("wv")
    bV = f32("wiv") @ f32("bv") + f32("biv")
    wo = f32("wo")
    bo = f32("bo")

    # multiplicative mask, transposed: maskT[m, n] = exp(add_mask[n, m])
    w = _softplus(etw)
    expw = np.exp(w).astype(np.float32)
    M = np.zeros((N, N), dtype=np.float32)
    src, dst = edge_index[0], edge_index[1]
    M[src, dst] = expw[edge_type - 1]          # last write wins, like jax .at[].set
    diag = np.diagonal(M).copy()
    didx = np.arange(N)
    M[didx, didx] = np.where(diag == 0.0, expw[3], diag)
    maskT = np.ascontiguousarray(M.T)
    if MASK_DT == BF16:
        import ml_dtypes
        maskT = maskT.astype(ml_dtypes.bfloat16)

    xT = np.ascontiguousarray(x.T)

    in_maps = []
    for c in range(NCORES):
        rs = slice(c * CW, (c + 1) * CW)
        in_maps.append({
            "xT": xT,
            "maskT": maskT,
            "wqT": np.ascontiguousarray(WQ[rs].T),
            "wkT": np.ascontiguousarray(WK[rs].T),
            "wvT": np.ascontiguousarray(WV[rs].T),
            "bq": np.ascontiguousarray(bQ[rs]),
            "bk": np.ascontiguousarray(bK[rs]),
            "bv": np.ascontiguousarray(bV[rs]),
            "woT": np.ascontiguousarray(wo[:, rs].T),
        })
    return in_maps, bo


def kernel(**inputs) -> np.ndarray:
    global LAST_RESULTS
    in_maps, bo = host_prep(inputs)
    nc = _get_program()
    trace = bool(os.environ.get("KERNEL_TRACE"))
    res = run_bass_kernel_spmd(nc, in_maps, list(range(NCORES)), trace=trace)
    LAST_RESULTS = res
    y = bo[None, :].astype(np.float32).repeat(N, axis=0)
    for c in range(NCORES):
        y += res.results[c]["ypart"]
    return y


# revision 17
# speedup vs baseline: 1.2091x; 1.2091x over previous
"""Sparse multi-head attention (nn_MultiHeadAttention_44332652429419) on 8 trn2 cores.

Strategy (tensor-parallel over H=16 heads, 2 heads per core):
  Host: compose the two stacked linear layers (q/k/v_proj followed by
        MultiheadAttention in_proj) into one weight per tensor; build the
        dense multiplicative mask exp(additive_mask) transposed; transpose x.
  Device (per core, SPMD with per-core weight slices):
    q2T/k2T/v2T = W_c @ x.T + b_c           [128, 3072] (2 heads x 64 dims)
    scoresT[mk,nq] = k2T_h.T-slice @ q2T_h  (K=64, two heads row-packed in PE)
    P = exp(scoresT * 1/8) * maskT          (ACT exp from PSUM, DVE multiply)
    outT_aug = [v_h | 1].T @ P              (rowsum via ones-augmented V)
    attnT = outT[:64] / outT[64]            (DVE recip + partition broadcast)
    ypart = attnT.T-slices @ woT_c          (partial out_proj, K=128)
  Host: y = sum_c ypart_c + bo
"""
import os
import sys

sys.path.insert(0, "/opt/trn_rl_repo")

import numpy as np
from contextlib import ExitStack

import concourse.bass as bass
import concourse.bacc as bacc
import concourse.mybir as mybir
import concourse.tile as tile
from concourse.bass_utils import run_bass_kernel_spmd
from concourse.masks import make_identity

F32 = mybir.dt.float32
F32R = mybir.dt.float32r
BF16 = mybir.dt.bfloat16
AF = mybir.ActivationFunctionType
ALU = mybir.AluOpType

N = 3072
IN_F = 1024
OUT_F = 1024
H = 16
D = 64
NCORES = 8
HPC = H // NCORES            # heads per core = 2
CW = HPC * D                 # per-core width = 128
P = 128
NT = N // P                  # 24 node tiles
KT = IN_F // P               # 8 contraction tiles
SQ = 1024                    # query strip width (phase B)
NSQ = N // SQ                # 3 strips
SP = 512                     # proj strip width (phase A)
NSP = N // SP                # 6 strips
SCALE = 1.0 / 8.0            # 1/sqrt(D)

MASK_DT = F32R               # additive mask, pre-scaled by 1/SCALE


def build_program():
    nc = bacc.Bacc()
    xT = nc.declare_dram_parameter("xT", [IN_F, N], F32R, isOutput=False)
    # additive mask (pre-scaled by 1/SCALE) for even key tiles, multiplicative
    # exp-mask for odd key tiles — hybrid PE/DVE mask application
    maskA = nc.declare_dram_parameter("maskA", [N // 2, N], F32R, isOutput=False)
    maskM = nc.declare_dram_parameter("maskM", [N // 2, N], F32, isOutput=False)
    wqT = nc.declare_dram_parameter("wqT", [IN_F, CW], F32R, isOutput=False)
    wkT = nc.declare_dram_parameter("wkT", [IN_F, CW], F32R, isOutput=False)
    wvT = nc.declare_dram_parameter("wvT", [IN_F, CW], F32R, isOutput=False)
    bq = nc.declare_dram_parameter("bq", [CW], F32, isOutput=False)
    bk = nc.declare_dram_parameter("bk", [CW], F32, isOutput=False)
    bv = nc.declare_dram_parameter("bv", [CW], F32, isOutput=False)
    woT = nc.declare_dram_parameter("woT", [CW, OUT_F], F32R, isOutput=False)
    ypart = nc.declare_dram_parameter("ypart", [N, OUT_F], F32, isOutput=True)

    with tile.TileContext(nc) as tc, ExitStack() as ctx:
        cst = ctx.enter_context(tc.tile_pool(name="cst", bufs=1))

        ident = cst.tile([P, P], F32)
        make_identity(nc, ident)
        identR = cst.tile([P, P], F32R)
        nc.vector.tensor_copy(identR[:], ident[:])

        # persistent SBUF tensors
        q2T = cst.tile([P, N], F32R)
        # per-head zero-padded K copies: k2z[h] has only rows h*D..h*D+63 live,
        # so score matmuls contract over the full K=128 (keeps the PE HAM warm)
        k2z = [cst.tile([P, N], F32R, tag=f"k2z{h}", name=f"k2z{h}")
               for h in range(HPC)]
        attnT = cst.tile([P, N], F32R)
        vaug = [cst.tile([P, NT, D + 1], F32R, tag=f"vaug{h}", name=f"vaug{h}")
                for h in range(HPC)]
        ones_col = cst.tile([P, 1], F32)
        nc.vector.memset(ones_col[:], 1.0)
        zero_col = cst.tile([P, 1], F32)
        nc.vector.memset(zero_col[:], 0.0)
        for h in range(HPC):
            nc.vector.tensor_copy(vaug[h][:, :, D:D + 1],
                                  ones_col[:, 0:1, None].to_broadcast([P, NT, 1]))
            osl = slice((1 - h) * D, (2 - h) * D)   # the dead half of k2z[h]
            nc.vector.tensor_copy(k2z[h][osl, :],
                                  zero_col[osl, 0:1].to_broadcast([D, N]))

        # weights
        wq_sb = cst.tile([P, KT, CW], F32R)
        nc.sync.dma_start(wq_sb[:], wqT.rearrange("(k p) m -> p k m", p=P))
        wk_sb = cst.tile([P, KT, CW], F32R)
        nc.sync.dma_start(wk_sb[:], wkT.rearrange("(k p) m -> p k m", p=P))
        wv_sb = cst.tile([P, KT, CW], F32R)
        nc.sync.dma_start(wv_sb[:], wvT.rearrange("(k p) m -> p k m", p=P))
        wo_sb = cst.tile([P, OUT_F], F32R)
        nc.sync.dma_start(wo_sb[:], woT[:])
        bq_sb = cst.tile([P, 1], F32)
        nc.sync.dma_start(bq_sb[:], bq[:, None])
        bk_sb = cst.tile([P, 1], F32)
        nc.sync.dma_start(bk_sb[:], bk[:, None])
        bv_sb = cst.tile([P, 1], F32)
        nc.sync.dma_start(bv_sb[:], bv[:, None])

        # ---- Phase A: projections ----
        with tc.tile_pool(name="pa_sb", bufs=2) as pa_sb, \
             tc.tile_pool(name="pa_ps", bufs=2, space="PSUM") as pa_ps:
            for s in range(NSP):
                sl = slice(s * SP, (s + 1) * SP)
                xs = pa_sb.tile([P, KT, SP], F32R, tag="xs")
                nc.sync.dma_start(
                    xs[:], xT.rearrange("(k p) n -> p k n", p=P)[:, :, sl])
                ps = pa_ps.tile([P, SP], F32, tag="ps_proj")
                for k in range(KT):
                    nc.tensor.matmul(ps[:], wq_sb[:, k, :], xs[:, k, :],
                                     start=(k == 0), stop=(k == KT - 1))
                nc.scalar.activation(q2T[:, sl], ps[:], AF.Identity,
                                     bias=bq_sb[:, 0:1])
                ps = pa_ps.tile([P, SP], F32, tag="ps_proj")
                for k in range(KT):
                    nc.tensor.matmul(ps[:], wk_sb[:, k, :], xs[:, k, :],
                                     start=(k == 0), stop=(k == KT - 1))
                for h in range(HPC):
                    hsl = slice(h * D, (h + 1) * D)
                    nc.scalar.activation(k2z[h][hsl, sl], ps[hsl, :], AF.Identity,
                                         bias=bk_sb[hsl, 0:1])
                # v: project then transpose into vaug
                ps = pa_ps.tile([P, SP], F32, tag="ps_proj")
                for k in range(KT):
                    nc.tensor.matmul(ps[:], wv_sb[:, k, :], xs[:, k, :],
                                     start=(k == 0), stop=(k == KT - 1))
                v2Ts = pa_sb.tile([P, SP], F32, tag="v2Ts")
                nc.scalar.activation(v2Ts[:], ps[:], AF.Identity, bias=bv_sb[:, 0:1])
                for b in range(SP // P):
                    t = s * (SP // P) + b
                    ps_t = pa_ps.tile([P, P], F32, tag="ps_t")
                    nc.tensor.transpose(ps_t[:], v2Ts[:, b * P:(b + 1) * P], ident[:])
                    for h in range(HPC):
                        nc.any.tensor_copy(vaug[h][:, t, 0:D],
                                           ps_t[:, h * D:(h + 1) * D])

        # ---- Phase B: attention + fused out_proj ----
        with tc.tile_pool(name="pb_sb", bufs=3) as pb_sb, \
             tc.tile_pool(name="pb_ps", bufs=2, space="PSUM") as pb_ps, \
             tc.tile_pool(name="pb_pso", bufs=1, space="PSUM") as pb_pso:
            for sq in range(NSQ):
                qsl = slice(sq * SQ, (sq + 1) * SQ)
                ps_o = [pb_pso.tile([D + 1, SQ], F32, tag=f"ps_o{h}", name=f"ps_o{h}")
                        for h in range(HPC)]
                for mk in range(NT):
                    use_pe = (mk % 2 == 0)
                    if use_pe:
                        mt = pb_sb.tile([P, SQ], F32R, tag="mta")
                        nc.sync.dma_start(
                            mt[:], maskA[(mk // 2) * P:(mk // 2 + 1) * P, qsl])
                    else:
                        mt = pb_sb.tile([P, SQ], F32, tag="mtm")
                        nc.sync.dma_start(
                            mt[:], maskM[(mk // 2) * P:(mk // 2 + 1) * P, qsl])
                    for h in range(HPC):
                        ps_s = pb_ps.tile([P, SQ], F32, tag="ps_s")
                        for half in range(SQ // 512):
                            fsl = slice(half * 512, (half + 1) * 512)
                            nc.tensor.matmul(
                                ps_s[:, fsl],
                                k2z[h][:, mk * P:(mk + 1) * P],
                                q2T[:, sq * SQ + half * 512:
                                    sq * SQ + (half + 1) * 512],
                                start=True, stop=not use_pe,
                            )
                            if use_pe:
                                nc.tensor.matmul(
                                    ps_s[:, fsl], identR[:], mt[:, fsl],
                                    start=False, stop=True,
                                )
                        p = pb_sb.tile([P, SQ], F32R, tag="p")
                        nc.scalar.activation(p[:], ps_s[:], AF.Exp, scale=SCALE)
                        if not use_pe:
                            pm = pb_sb.tile([P, SQ], F32R, tag="pm")
                            nc.vector.tensor_tensor(pm[:], p[:], mt[:], ALU.mult)
                            p = pm
                        for half in range(SQ // 512):
                            fsl = slice(half * 512, (half + 1) * 512)
                            nc.tensor.matmul(
                                ps_o[h][:, fsl],
                                vaug[h][:, mk, :],
                                p[:, fsl],
                                start=(mk == 0), stop=(mk == NT - 1),
                            )
                for h in range(HPC):
                    zrow = pb_sb.tile([1, SQ], F32, tag="zrow")
                    nc.vector.tensor_copy(zrow[:], ps_o[h][D:D + 1, :])
                    recip = pb_sb.tile([1, SQ], F32, tag="recip")
                    nc.vector.reciprocal_approx_fast(recip[:], zrow[:])
                    bc = pb_sb.tile([D, SQ], F32, tag="bc")
                    nc.gpsimd.partition_broadcast(bc[:], recip[:])
                    nc.vector.tensor_tensor(attnT[h * D:(h + 1) * D, qsl],
                                            ps_o[h][0:D, :], bc[:], ALU.mult)
                # fused partial out_proj for this strip's node tiles
                for t in range(sq * (SQ // P), (sq + 1) * (SQ // P)):
                    for f in range(OUT_F // 512):
                        ps_y = pb_ps.tile([P, SQ], F32, tag="ps_s", name="ps_y")
                        nc.tensor.matmul(ps_y[:, 0:512],
                                         attnT[:, t * P:(t + 1) * P],
                                         wo_sb[:, f * 512:(f + 1) * 512],
                                         start=True, stop=True)
                        ys = pb_sb.tile([P, 512], F32, tag="ys")
                        nc.vector.tensor_copy(ys[:], ps_y[:, 0:512])
                        nc.sync.dma_start(
                            ypart[t * P:(t + 1) * P, f * 512:(f + 1) * 512],
                            ys[:])

    nc.compile()
    return nc


_PROGRAM = None
LAST_RESULTS = None


def _get_program():
    global _PROGRAM
    if _PROGRAM is None:
        _PROGRAM = build_program()
    return _PROGRAM


def _softplus(x):
    x = np.asarray(x, np.float32)
    return np.logaddexp(0.0, x).astype(np.float32)


def host_prep(inputs):
    x = np.asarray(inputs["x"], np.float32)
    edge_index = np.asarray(inputs["edge_index"])
    edge_type = np.asarray(inputs["edge_type"])
    etw = np.asarray(inputs["edge_type_weights"], np.float32)

    def f32(k):
        return np.asarray(inputs[k], np.float32)

    # compose the two linear layers: q2 = x @ (wiq@wq).T + (wiq@bq + biq)
    WQ = f32("wiq") @ f32("wq")
    bQ = f32("wiq") @ f32("bq") + f32("biq")
    WK = f32("wik") @ f32("wk")
    bK = f32("wik") @ f32("bk") + f32("bik")
    WV = f32("wiv") @ f32("wv")
    bV = f32("wiv") @ f32("bv") + f32("biv")
    wo = f32("wo")
    bo = f32("bo")

    # multiplicative mask, transposed: maskT[m, n] = exp(add_mask[n, m])
    w = _softplus(etw)
    NEG = np.float32(-8e30)
    M = np.full((N, N), NEG, dtype=np.float32)
    src, dst = edge_index[0], edge_index[1]
    wv8 = (w * np.float32(1.0 / SCALE)).astype(np.float32)
    M[src, dst] = wv8[edge_type - 1]           # last write wins, like jax .at[].set
    diag = np.diagonal(M).copy()
    didx = np.arange(N)
    M[didx, didx] = np.where(diag == NEG, wv8[3], diag)
    MT = np.ascontiguousarray(M.T)             # [key m, query n], additive * 8
    # even key tiles use the additive form on the PE, odd tiles the
    # multiplicative exp-form on the DVE
    MT4 = MT.reshape(NT, P, N)
    maskA = np.ascontiguousarray(MT4[0::2].reshape(N // 2, N))
    maskM = np.exp(MT4[1::2].reshape(N // 2, N).astype(np.float64)
                   * np.float64(SCALE)).astype(np.float32)

    xT = np.ascontiguousarray(x.T)

    in_maps = []
    for c in range(NCORES):
        rs = slice(c * CW, (c + 1) * CW)
        in_maps.append({
            "xT": xT,
            "maskA": maskA,
            "maskM": maskM,
            "wqT": np.ascontiguousarray(WQ[rs].T),
            "wkT": np.ascontiguousarray(WK[rs].T),
            "wvT": np.ascontiguousarray(WV[rs].T),
            "bq": np.ascontiguousarray(bQ[rs]),
            "bk": np.ascontiguousarray(bK[rs]),
            "bv": np.ascontiguousarray(bV[rs]),
            "woT": np.ascontiguousarray(wo[:, rs].T),
        })
    return in_maps, bo


def kernel(**inputs) -> np.ndarray:
    global LAST_RESULTS
    in_maps, bo = host_prep(inputs)
    nc = _get_program()
    trace = bool(os.environ.get("KERNEL_TRACE"))
    res = run_bass_kernel_spmd(nc, in_maps, list(range(NCORES)), trace=trace)
    LAST_RESULTS = res
    y = bo[None, :].astype(np.float32).repeat(N, axis=0)
    for c in range(NCORES):
        y += res.results[c]["ypart"]
    return y


# revision 19
# speedup vs baseline: 1.2621x; 1.0439x over previous
"""Sparse multi-head attention (nn_MultiHeadAttention_44332652429419) on 8 trn2 cores.

Strategy (tensor-parallel over H=16 heads, 2 heads per core):
  Host: compose the two stacked linear layers (q/k/v_proj followed by
        MultiheadAttention in_proj) into one weight per tensor; build the
        dense multiplicative mask exp(additive_mask) transposed; transpose x.
  Device (per core, SPMD with per-core weight slices):
    q2T/k2T/v2T = W_c @ x.T + b_c           [128, 3072] (2 heads x 64 dims)
    scoresT[mk,nq] = k2T_h.T-slice @ q2T_h  (K=64, two heads row-packed in PE)
    P = exp(scoresT * 1/8) * maskT          (ACT exp from PSUM, DVE multiply)
    outT_aug = [v_h | 1].T @ P              (rowsum via ones-augmented V)
    attnT = outT[:64] / outT[64]            (DVE recip + partition broadcast)
    ypart = attnT.T-slices @ woT_c          (partial out_proj, K=128)
  Host: y = sum_c ypart_c + bo
"""
import os
import sys

sys.path.insert(0, "/opt/trn_rl_repo")

import numpy as np
from contextlib import ExitStack

import concourse.bass as bass
import concourse.bacc as bacc
import concourse.mybir as mybir
import concourse.tile as tile
from concourse.bass_utils import run_bass_kernel_spmd
from concourse.masks import make_identity

F32 = mybir.dt.float32
F32R = mybir.dt.float32r
BF16 = mybir.dt.bfloat16
AF = mybir.ActivationFunctionType
ALU = mybir.AluOpType

N = 3072
IN_F = 1024
OUT_F = 1024
H = 16
D = 64
NCORES = 8
HPC = H // NCORES            # heads per core = 2
CW = HPC * D                 # per-core width = 128
P = 128
NT = N // P                  # 24 node tiles
KT = IN_F // P               # 8 contraction tiles
SQ = 1024                    # query strip width (phase B)
NSQ = N // SQ                # 3 strips
SP = 512                     # proj strip width (phase A)
NSP = N // SP                # 6 strips
SCALE = 1.0 / 8.0            # 1/sqrt(D)

MASK_DT = F32R               # additive mask, pre-scaled by 1/SCALE


def build_program():
    nc = bacc.Bacc()
    xT = nc.declare_dram_parameter("xT", [IN_F, N], F32R, isOutput=False)
    # additive mask (pre-scaled by 1/SCALE) for even key tiles, multiplicative
    # exp-mask for odd key tiles — hybrid PE/DVE mask application
    maskA = nc.declare_dram_parameter("maskA", [N // 2, N], F32R, isOutput=False)
    maskM = nc.declare_dram_parameter("maskM", [N // 2, N], F32, isOutput=False)
    wqT = nc.declare_dram_parameter("wqT", [IN_F, CW], F32R, isOutput=False)
    wkT = nc.declare_dram_parameter("wkT", [IN_F, CW], F32R, isOutput=False)
    wvT = nc.declare_dram_parameter("wvT", [IN_F, CW], F32R, isOutput=False)
    bq = nc.declare_dram_parameter("bq", [CW], F32, isOutput=False)
    bk = nc.declare_dram_parameter("bk", [CW], F32, isOutput=False)
    bv = nc.declare_dram_parameter("bv", [CW], F32, isOutput=False)
    woT = nc.declare_dram_parameter("woT", [CW, OUT_F], F32R, isOutput=False)
    ypart = nc.declare_dram_parameter("ypart", [N, OUT_F], F32, isOutput=True)

    with tile.TileContext(nc) as tc, ExitStack() as ctx:
        cst = ctx.enter_context(tc.tile_pool(name="cst", bufs=1))

        ident = cst.tile([P, P], F32)
        make_identity(nc, ident)
        identR = cst.tile([P, P], F32R)
        nc.vector.tensor_copy(identR[:], ident[:])

        # persistent SBUF tensors
        q2T = cst.tile([P, N], F32R)
        # per-head zero-padded K copies: k2z[h] has only rows h*D..h*D+63 live,
        # so score matmuls contract over the full K=128 (keeps the PE HAM warm)
        k2z = [cst.tile([P, N], F32R, tag=f"k2z{h}", name=f"k2z{h}")
               for h in range(HPC)]
        attnT = cst.tile([P, N], F32R)
        vaug = [cst.tile([P, NT, D + 1], F32R, tag=f"vaug{h}", name=f"vaug{h}")
                for h in range(HPC)]
        ones_col = cst.tile([P, 1], F32)
        nc.vector.memset(ones_col[:], 1.0)
        zero_col = cst.tile([P, 1], F32)
        nc.vector.memset(zero_col[:], 0.0)
        for h in range(HPC):
            nc.vector.tensor_copy(vaug[h][:, :, D:D + 1],
                                  ones_col[:, 0:1, None].to_broadcast([P, NT, 1]))
            osl = slice((1 - h) * D, (2 - h) * D)   # the dead half of k2z[h]
            nc.vector.tensor_copy(k2z[h][osl, :],
                                  zero_col[osl, 0:1].to_broadcast([D, N]))

        # weights
        wq_sb = cst.tile([P, KT, CW], F32R)
        nc.sync.dma_start(wq_sb[:], wqT.rearrange("(k p) m -> p k m", p=P))
        wk_sb = cst.tile([P, KT, CW], F32R)
        nc.sync.dma_start(wk_sb[:], wkT.rearrange("(k p) m -> p k m", p=P))
        wv_sb = cst.tile([P, KT, CW], F32R)
        nc.sync.dma_start(wv_sb[:], wvT.rearrange("(k p) m -> p k m", p=P))
        wo_sb = cst.tile([P, OUT_F], F32R)
        nc.sync.dma_start(wo_sb[:], woT[:])
        bq_sb = cst.tile([P, 1], F32)
        nc.sync.dma_start(bq_sb[:], bq[:, None])
        bk_sb = cst.tile([P, 1], F32)
        nc.sync.dma_start(bk_sb[:], bk[:, None])
        bv_sb = cst.tile([P, 1], F32)
        nc.sync.dma_start(bv_sb[:], bv[:, None])

        # ---- Phase A: projections ----
        with tc.tile_pool(name="pa_sb", bufs=2) as pa_sb, \
             tc.tile_pool(name="pa_ps", bufs=2, space="PSUM") as pa_ps:
            for s in range(NSP):
                sl = slice(s * SP, (s + 1) * SP)
                xs = pa_sb.tile([P, KT, SP], F32R, tag="xs")
                nc.sync.dma_start(
                    xs[:], xT.rearrange("(k p) n -> p k n", p=P)[:, :, sl])
                ps = pa_ps.tile([P, SP], F32, tag="ps_proj")
                for k in range(KT):
                    nc.tensor.matmul(ps[:], wq_sb[:, k, :], xs[:, k, :],
                                     start=(k == 0), stop=(k == KT - 1))
                nc.scalar.activation(q2T[:, sl], ps[:], AF.Identity,
                                     bias=bq_sb[:, 0:1])
                ps = pa_ps.tile([P, SP], F32, tag="ps_proj")
                for k in range(KT):
                    nc.tensor.matmul(ps[:], wk_sb[:, k, :], xs[:, k, :],
                                     start=(k == 0), stop=(k == KT - 1))
                for h in range(HPC):
                    hsl = slice(h * D, (h + 1) * D)
                    nc.scalar.activation(k2z[h][hsl, sl], ps[hsl, :], AF.Identity,
                                         bias=bk_sb[hsl, 0:1])
                # v: project then transpose into vaug
                ps = pa_ps.tile([P, SP], F32, tag="ps_proj")
                for k in range(KT):
                    nc.tensor.matmul(ps[:], wv_sb[:, k, :], xs[:, k, :],
                                     start=(k == 0), stop=(k == KT - 1))
                v2Ts = pa_sb.tile([P, SP], F32, tag="v2Ts")
                nc.scalar.activation(v2Ts[:], ps[:], AF.Identity, bias=bv_sb[:, 0:1])
                for b in range(SP // P):
                    t = s * (SP // P) + b
                    ps_t = pa_ps.tile([P, P], F32, tag="ps_t")
                    nc.tensor.transpose(ps_t[:], v2Ts[:, b * P:(b + 1) * P], ident[:])
                    for h in range(HPC):
                        nc.any.tensor_copy(vaug[h][:, t, 0:D],
                                           ps_t[:, h * D:(h + 1) * D])

        # ---- Phase B: attention + fused out_proj ----
        with tc.tile_pool(name="pb_sb", bufs=3) as pb_sb, \
             tc.tile_pool(name="pb_ep", bufs=2) as pb_ep, \
             tc.tile_pool(name="pb_ps", bufs=2, space="PSUM") as pb_ps, \
             tc.tile_pool(name="pb_pso", bufs=1, space="PSUM") as pb_pso:
            for sq in range(NSQ):
                qsl = slice(sq * SQ, (sq + 1) * SQ)
                ps_o = [pb_pso.tile([D + 1, SQ], F32, tag=f"ps_o{h}", name=f"ps_o{h}")
                        for h in range(HPC)]
                for mk in range(NT):
                    use_pe = (mk % 2 == 0)
                    if use_pe:
                        mt = pb_sb.tile([P, SQ], F32R, tag="mta")
                        nc.sync.dma_start(
                            mt[:], maskA[(mk // 2) * P:(mk // 2 + 1) * P, qsl])
                    else:
                        mt = pb_sb.tile([P, SQ], F32, tag="mtm")
                        nc.sync.dma_start(
                            mt[:], maskM[(mk // 2) * P:(mk // 2 + 1) * P, qsl])
                    for h in range(HPC):
                        ps_s = pb_ps.tile([P, SQ], F32, tag="ps_s")
                        for half in range(SQ // 512):
                            fsl = slice(half * 512, (half + 1) * 512)
                            nc.tensor.matmul(
                                ps_s[:, fsl],
                                k2z[h][:, mk * P:(mk + 1) * P],
                                q2T[:, sq * SQ + half * 512:
                                    sq * SQ + (half + 1) * 512],
                                start=True, stop=not use_pe,
                            )
                            if use_pe:
                                nc.tensor.matmul(
                                    ps_s[:, fsl], identR[:], mt[:, fsl],
                                    start=False, stop=True,
                                )
                        p = pb_sb.tile([P, SQ], F32R, tag="p")
                        nc.scalar.activation(p[:], ps_s[:], AF.Exp, scale=SCALE)
                        if not use_pe:
                            pm = pb_sb.tile([P, SQ], F32R, tag="pm")
                            nc.vector.tensor_tensor(pm[:], p[:], mt[:], ALU.mult)
                            p = pm
                        for half in range(SQ // 512):
                            fsl = slice(half * 512, (half + 1) * 512)
                            nc.tensor.matmul(
                                ps_o[h][:, fsl],
                                vaug[h][:, mk, :],
                                p[:, fsl],
                                start=(mk == 0), stop=(mk == NT - 1),
                            )
                bcs = []
                for h in range(HPC):
                    zrow = pb_ep.tile([1, SQ], F32, tag=f"zrow{h}", name=f"zrow{h}")
                    nc.vector.tensor_copy(zrow[:], ps_o[h][D:D + 1, :])
                    recip = pb_ep.tile([1, SQ], F32, tag=f"recip{h}",
                                       name=f"recip{h}")
                    nc.vector.reciprocal_approx_fast(recip[:], zrow[:])
                    bc = pb_ep.tile([D, SQ], F32, tag=f"bc{h}", name=f"bc{h}")
                    nc.gpsimd.partition_broadcast(bc[:], recip[:])
                    bcs.append(bc)
                # normalize per node tile so out_proj can start early
                for b in range(SQ // P):
                    for h in range(HPC):
                        nc.vector.tensor_tensor(
                            attnT[h * D:(h + 1) * D,
                                  sq * SQ + b * P:sq * SQ + (b + 1) * P],
                            ps_o[h][0:D, b * P:(b + 1) * P],
                            bcs[h][:, b * P:(b + 1) * P], ALU.mult)
                    t = sq * (SQ // P) + b
                    ps_y = pb_ps.tile([P, SQ], F32, tag="ps_s", name="ps_y")
                    for f in range(OUT_F // 512):
                        nc.tensor.matmul(ps_y[:, f * 512:(f + 1) * 512],
                                         attnT[:, t * P:(t + 1) * P],
                                         wo_sb[:, f * 512:(f + 1) * 512],
                                         start=True, stop=True)
                    ys = pb_ep.tile([P, OUT_F], F32, tag="ys")
                    nc.any.tensor_copy(ys[:], ps_y[:])
                    nc.sync.dma_start(ypart[t * P:(t + 1) * P, :], ys[:])

    nc.compile()
    return nc


_PROGRAM = None
LAST_RESULTS = None


def _get_program():
    global _PROGRAM
    if _PROGRAM is None:
        _PROGRAM = build_program()
    return _PROGRAM


def _softplus(x):
    x = np.asarray(x, np.float32)
    return np.logaddexp(0.0, x).astype(np.float32)


def host_prep(inputs):
    x = np.asarray(inputs["x"], np.float32)
    edge_index = np.asarray(inputs["edge_index"])
    edge_type = np.asarray(inputs["edge_type"])
    etw = np.asarray(inputs["edge_type_weights"], np.float32)

    def f32(k):
        return np.asarray(inputs[k], np.float32)

    # compose the two linear layers: q2 = x @ (wiq@wq).T + (wiq@bq + biq)
    WQ = f32("wiq") @ f32("wq")
    bQ = f32("wiq") @ f32("bq") + f32("biq")
    WK = f32("wik") @ f32("wk")
    bK = f32("wik") @ f32("bk") + f32("bik")
    WV = f32("wiv") @ f32("wv")
    bV = f32("wiv") @ f32("bv") + f32("biv")
    wo = f32("wo")
    bo = f32("bo")

    # multiplicative mask, transposed: maskT[m, n] = exp(add_mask[n, m])
    w = _softplus(etw)
    NEG = np.float32(-8e30)
    M = np.full((N, N), NEG, dtype=np.float32)
    src, dst = edge_index[0], edge_index[1]
    wv8 = (w * np.float32(1.0 / SCALE)).astype(np.float32)
    M[src, dst] = wv8[edge_type - 1]           # last write wins, like jax .at[].set
    diag = np.diagonal(M).copy()
    didx = np.arange(N)
    M[didx, didx] = np.where(diag == NEG, wv8[3], diag)
    MT = np.ascontiguousarray(M.T)             # [key m, query n], additive * 8
    # even key tiles use the additive form on the PE, odd tiles the
    # multiplicative exp-form on the DVE
    MT4 = MT.reshape(NT, P, N)
    maskA = np.ascontiguousarray(MT4[0::2].reshape(N // 2, N))
    maskM = np.exp(MT4[1::2].reshape(N // 2, N).astype(np.float64)
                   * np.float64(SCALE)).astype(np.float32)

    xT = np.ascontiguousarray(x.T)

    in_maps = []
    for c in range(NCORES):
        rs = slice(c * CW, (c + 1) * CW)
        in_maps.append({
            "xT": xT,
            "maskA": maskA,
            "maskM": maskM,
            "wqT": np.ascontiguousarray(WQ[rs].T),
            "wkT": np.ascontiguousarray(WK[rs].T),
            "wvT": np.ascontiguousarray(WV[rs].T),
            "bq": np.ascontiguousarray(bQ[rs]),
            "bk": np.ascontiguousarray(bK[rs]),
            "bv": np.ascontiguousarray(bV[rs]),
            "woT": np.ascontiguousarray(wo[:, rs].T),
        })
    return in_maps, bo


def kernel(**inputs) -> np.ndarray:
    global LAST_RESULTS
    in_maps, bo = host_prep(inputs)
    nc = _get_program()
    trace = bool(os.environ.get("KERNEL_TRACE"))
    res = run_bass_kernel_spmd(nc, in_maps, list(range(NCORES)), trace=trace)
    LAST_RESULTS = res
    y = bo[None, :].astype(np.float32).repeat(N, axis=0)
    for c in range(NCORES):
        y += res.results[c]["ypart"]
    return y
